# Initial kernel scaffold
#
"""Trainium2 Bass kernel for masked dot-product attention.

Problem: B=16, Lq=Lk=2048, d=128, fp32.
  scores = Q @ K^T / sqrt(d); mask key positions >= valid_len with -1e6;
  attn = softmax(scores, axis=-1); out = attn @ V.

Strategy
--------
The work is sharded over (batch, query-quarter): 16 batches x 4 q-chunks of
512 = 64 shards, 8 per core. A shard's device cost is proportional to
ceil(valid_len/128) key tiles, so shards are sorted by tile count and slot s
of every core runs the 8 shards ranked [8s, 8s+8); the compiled program bakes
per-slot key extents E_s = max tile count in that rank band. Device work thus
scales with the actual valid lengths (~2x less than processing all keys),
and every core executes an identical instruction stream (SPMD).

All layout work happens on the host inside kernel():
  * Q^T, K^T ([d, L], d on partitions) are prepared with numpy, so the device
    does zero transposes.
  * Masking is exact and host-side: V rows at k >= valid_len are zeroed and a
    0/1 vector z replaces the "ones" column of the softmax denominator. exp()
    never sees masked scores, so no -1e6 arithmetic happens on device.
  * Matmul operands are pre-rounded to fp32r (11-bit mantissa) on the host so
    the PE streams 1 column/cycle (plain fp32 is 4x slower).

Device program per slot (one 512-wide q-chunk, E_s key tiles):
  MM1:  S^T[k,q] = (K^T tile).T @ Q^T     (PE, fp32r, N=512, k-tile stationary)
  exp:  E = exp(S^T / sqrt(d))            (ACT, PSUM->SBUF fp32r, 2 k-tiles/pass)
  MM2:  num^T[d,q] += lhsT=V_tile[k,d] -> sum_k V[k,d]*E[k,q]   (PE accumulate)
  den:  den[q]     += z_tile.T @ E        (PE, lhsT=[128,1])
MM1/exp run one k-group ahead of MM2/den (software pipeline; psum double
buffered) so PE and ACT overlap. Host computes out = (num^T / den).T per
shard. Softmax needs no max-subtraction: scores ~ N(0,1), exp() cannot
overflow fp32, and masked columns contribute exactly zero.
"""

import math

import numpy as np

B, L, D = 16, 2048, 128
NCORES = 8
QCHUNK = 512
NQCHUNKS = L // QCHUNK
NSLOTS = B * NQCHUNKS // NCORES  # 8
GSZ = 2  # k-tiles per exp group; [128, GSZ*512] psum tiles, double-buffered
SCALE = 1.0 / math.sqrt(D)

_programs = {}

# Test hooks: _REPEAT>1 duplicates the whole slot schedule inside one NEFF
# (for wall-clock-delta timing); _last_results holds the raw results.
_TRACE = False
_REPEAT = 1
_last_results = None


def _round_f32r(arr):
    """Round-to-nearest-even fp32 -> fp32r (11-bit mantissa, low 12 bits zero)."""
    bits = np.ascontiguousarray(arr, dtype=np.float32).view(np.uint32)
    keep = bits & np.uint32(0xFFFFF000)
    rem = bits & np.uint32(0x00000FFF)
    lsb = (bits >> np.uint32(12)) & np.uint32(1)
    roundup = (rem > 0x800) | ((rem == 0x800) & (lsb == 1))
    return (keep + (roundup.astype(np.uint32) << np.uint32(12))).view(np.float32)


def _build_program(extents, repeat=1):
    import concourse.tile as tile
    from concourse import bacc, mybir

    F32 = mybir.dt.float32
    F32R = mybir.dt.float32r
    Tmax = max(extents)

    nc = bacc.Bacc("TRN2")

    ins = {}
    outs = {}
    for s, T in enumerate(extents):
        ins[f"qt{s}"] = nc.dram_tensor(f"qt{s}", [128, QCHUNK], F32R, kind="ExternalInput")
        ins[f"kt{s}"] = nc.dram_tensor(f"kt{s}", [128, T * 128], F32R, kind="ExternalInput")
        ins[f"v{s}"] = nc.dram_tensor(f"v{s}", [128, T * 128], F32R, kind="ExternalInput")
        ins[f"z{s}"] = nc.dram_tensor(f"z{s}", [128, T], F32R, kind="ExternalInput")
        outs[f"num{s}"] = nc.dram_tensor(f"num{s}", [128, QCHUNK], F32, kind="ExternalOutput")
        outs[f"den{s}"] = nc.dram_tensor(f"den{s}", [1, QCHUNK], F32, kind="ExternalOutput")

    with tile.TileContext(nc) as tc:
        with (
            tc.tile_pool(name="inp", bufs=3) as inp,
            tc.tile_pool(name="epool", bufs=3) as epool,
            tc.tile_pool(name="opool", bufs=3) as opool,
            tc.tile_pool(name="dpool", bufs=3) as dpool,
            tc.tile_pool(name="ps_s", bufs=2, space="PSUM") as ps_s,
            tc.tile_pool(name="ps_o", bufs=2, space="PSUM") as ps_o,
            tc.tile_pool(name="ps_d", bufs=2, space="PSUM") as ps_d,
        ):
            for s, T in [(s, T) for _ in range(repeat) for s, T in enumerate(extents)]:
                qt = inp.tile([128, QCHUNK], F32R, tag="qt")
                kt = inp.tile([128, Tmax * 128], F32R, tag="kt")
                vt = inp.tile([128, Tmax * 128], F32R, tag="vt")
                zt = inp.tile([128, Tmax], F32R, tag="zt")
                nc.sync.dma_start(out=kt[:, : T * 128], in_=ins[f"kt{s}"][:, :])
                nc.sync.dma_start(out=qt, in_=ins[f"qt{s}"][:, :])
                nc.sync.dma_start(out=vt[:, : T * 128], in_=ins[f"v{s}"][:, :])
                nc.sync.dma_start(out=zt[:, :T], in_=ins[f"z{s}"][:, :])
                # [128,1]-stationary matmuls are pathologically slow on HW
                # (~70us each), so the denominator matmul uses a full 128-col
                # stationary: z broadcast along the free dim (every output row
                # then holds the denominator; row 0 is copied out).
                zr = inp.tile([128, Tmax * 128], F32R, tag="zr")
                for t in range(T):
                    nc.vector.tensor_copy(
                        zr[:, t * 128 : (t + 1) * 128],
                        zt[:, t : t + 1].to_broadcast([128, 128]),
                    )

                ngroups = (T + GSZ - 1) // GSZ
                po = ps_o.tile([128, QCHUNK], F32, tag="po")
                pd = ps_d.tile([128, QCHUNK], F32, tag="pd")
                # MM1/exp of group g run one group ahead of MM2/den of g-1:
                # the PE issues the next group's MM1 (feeding ACT) before
                # draining the previous group's consumers.
                pending = None
                for g in range(ngroups + 1):
                    if g < ngroups:
                        gtiles = list(range(g * GSZ, min(g * GSZ + GSZ, T)))
                        gn = len(gtiles)
                        pss = ps_s.tile([128, GSZ * QCHUNK], F32, tag="ps")
                        for j, t in enumerate(gtiles):
                            nc.tensor.matmul(
                                pss[:, j * QCHUNK : (j + 1) * QCHUNK],
                                kt[:, t * 128 : (t + 1) * 128],
                                qt,
                                start=True,
                                stop=True,
                            )
                        eg = epool.tile([128, GSZ * QCHUNK], F32R, tag="eg")
                        nc.scalar.activation(
                            eg[:, : gn * QCHUNK],
                            pss[:, : gn * QCHUNK],
                            mybir.ActivationFunctionType.Exp,
                            scale=SCALE,
                        )
                        cur = (gtiles, eg)
                    else:
                        cur = None
                    if pending is not None:
                        ptiles, peg = pending
                        for j, t in enumerate(ptiles):
                            es = peg[:, j * QCHUNK : (j + 1) * QCHUNK]
                            nc.tensor.matmul(
                                po,
                                vt[:, t * 128 : (t + 1) * 128],
                                es,
                                start=(t == 0),
                                stop=(t == T - 1),
                            )
                            nc.tensor.matmul(
                                pd,
                                zr[:, t * 128 : (t + 1) * 128],
                                es,
                                start=(t == 0),
                                stop=(t == T - 1),
                            )
                    pending = cur
                osb = opool.tile([128, QCHUNK], F32, tag="osb")
                nc.vector.tensor_copy(osb, po)
                nc.sync.dma_start(out=outs[f"num{s}"][:, :], in_=osb)
                dsb = dpool.tile([1, QCHUNK], F32, tag="dsb")
                nc.vector.tensor_copy(dsb, pd[0:1, :])
                nc.sync.dma_start(out=outs[f"den{s}"][:, :], in_=dsb)

    nc.finalize()
    return nc


def _get_program(extents, repeat=1):
    key = (tuple(extents), repeat)
    if key not in _programs:
        _programs[key] = _build_program(tuple(extents), repeat)
    return _programs[key]


def _shard_plan(vl):
    """64 (batch, q-chunk) shards sorted by key-tile count desc; slot s of
    core c runs shard rank s*8+c. Returns (shards, extents)."""
    tiles = [max(1, int(math.ceil(int(vl[b]) / 128.0))) for b in range(B)]
    shards = sorted(
        ((tiles[b], b, qc) for b in range(B) for qc in range(NQCHUNKS)),
        key=lambda x: (-x[0], x[1], x[2]),
    )
    extents = tuple(shards[s * NCORES][0] for s in range(NSLOTS))
    return shards, extents


def _make_in_maps(queries, keys, values, vl, shards, extents):
    # kt/vt/zt depend only on (batch, extent): memoize across the 4 q-shards
    kcache = {}

    def kvz(b, T):
        key = (b, T)
        if key not in kcache:
            n = int(vl[b])
            vs = values[b, : T * 128].copy()
            vs[n:] = 0.0
            z = np.zeros((T * 128,), np.float32)
            z[:n] = 1.0
            kcache[key] = (
                _round_f32r(keys[b, : T * 128].T),
                _round_f32r(vs.reshape(T, 128, D).transpose(1, 0, 2).reshape(128, T * D)),
                np.ascontiguousarray(z.reshape(T, 128).T),
            )
        return kcache[key]

    qtr = {}  # rounded Q^T per batch

    def qtb(b):
        if b not in qtr:
            qtr[b] = _round_f32r(queries[b].T)
        return qtr[b]

    in_maps = [{} for _ in range(NCORES)]
    for s in range(NSLOTS):
        T = extents[s]
        for c in range(NCORES):
            _, b, qc = shards[s * NCORES + c]
            kt, vt, zt = kvz(b, T)
            m = in_maps[c]
            m[f"qt{s}"] = np.ascontiguousarray(
                qtb(b)[:, qc * QCHUNK : (qc + 1) * QCHUNK]
            )
            m[f"kt{s}"] = kt
            m[f"v{s}"] = vt
            m[f"z{s}"] = zt
    return in_maps


def kernel(queries, keys, values, valid_lens):
    from concourse.bass_utils import run_bass_kernel_spmd

    queries = np.ascontiguousarray(queries, dtype=np.float32)
    keys = np.ascontiguousarray(keys, dtype=np.float32)
    values = np.ascontiguousarray(values, dtype=np.float32)
    vl = np.asarray(valid_lens).astype(np.int64).clip(1, L)
    assert queries.shape == (B, L, D), queries.shape

    shards, extents = _shard_plan(vl)
    nc = _get_program(extents, _REPEAT)
    in_maps = _make_in_maps(queries, keys, values, vl, shards, extents)

    res = run_bass_kernel_spmd(nc, in_maps, core_ids=list(range(NCORES)), trace=_TRACE)
    globals()["_last_results"] = res

    out = np.empty((B, L, D), np.float32)
    for s in range(NSLOTS):
        for c in range(NCORES):
            _, b, qc = shards[s * NCORES + c]
            r = res.results[c]
            num = r[f"num{s}"]  # [128, QCHUNK]
            den = r[f"den{s}"]  # [1, QCHUNK]
            out[b, qc * QCHUNK : (qc + 1) * QCHUNK] = (num / den).T
    return out



# revision 2
# speedup vs baseline: 1.3858x; 1.3858x over previous
"""Trainium2 Bass kernel for masked dot-product attention (v5).

v5 changes vs v4:
  * Shards are (batch, half) — 32 shards of 1024 query rows, 4 slots of 8.
    Each slot DMAs K^T/V once and runs TWO sequential 512-query pipeline
    passes over them, halving the K/V upload bytes vs quarter sharding at
    the cost of coarser load-balance (padded tile sum 30 vs 25 per core).
  * bf16 num outputs (halves output bytes), f32 den.
"""

import math

import numpy as np

B, L, D = 16, 2048, 128
NCORES = 8
QCHUNK = 512
HCHUNK = 1024
NHALF = L // HCHUNK  # 2
NSLOTS = B * NHALF // NCORES  # 4
GSZ = 2
SCALE = 1.0 / math.sqrt(D)

_programs = {}

_TRACE = False
_REPEAT = 1
_last_results = None


def _build_program(extents, mins, repeat=1):
    import concourse.tile as tile
    from concourse import bacc, mybir

    F32 = mybir.dt.float32
    BF16 = mybir.dt.bfloat16
    Tmax = max(extents)

    nc = bacc.Bacc("TRN2")

    ins = {}
    outs = {}
    for s, T in enumerate(extents):
        nmask = T - max(0, mins[s] - 1)
        W = HCHUNK + 2 * T * 128 + nmask
        ins[f"in{s}"] = nc.dram_tensor(f"in{s}", [128, W], BF16, kind="ExternalInput")
        for qc in range(2):
            o = 2 * s + qc
            outs[f"num{o}"] = nc.dram_tensor(f"num{o}", [128, QCHUNK], BF16, kind="ExternalOutput")
            outs[f"den{o}"] = nc.dram_tensor(f"den{o}", [1, QCHUNK], F32, kind="ExternalOutput")

    with tile.TileContext(nc) as tc:
        with (
            tc.tile_pool(name="const", bufs=1) as const,
            tc.tile_pool(name="inp", bufs=2) as inp,
            tc.tile_pool(name="epool", bufs=3) as epool,
            tc.tile_pool(name="gpool", bufs=3) as gpool,
            tc.tile_pool(name="opool", bufs=3) as opool,
            tc.tile_pool(name="dpool", bufs=3) as dpool,
            tc.tile_pool(name="ps_s", bufs=2, space="PSUM") as ps_s,
            tc.tile_pool(name="ps_o", bufs=2, space="PSUM") as ps_o,
            tc.tile_pool(name="ps_d", bufs=2, space="PSUM") as ps_d,
        ):
            ones = const.tile([128, 128], BF16, tag="ones")
            nc.vector.memset(ones, 1.0)
            Wmax = HCHUNK + 2 * Tmax * 128 + Tmax
            for s, T in [(s, T) for _ in range(repeat) for s, T in enumerate(extents)]:
                mn = mins[s]
                nmask = T - max(0, mn - 1)
                W = HCHUNK + 2 * T * 128 + nmask
                it = inp.tile([128, Wmax], BF16, tag="it")
                nc.sync.dma_start(out=it[:, :W], in_=ins[f"in{s}"][:, :])
                kt = it[:, HCHUNK : HCHUNK + T * 128]
                vt = it[:, HCHUNK + T * 128 : HCHUNK + 2 * T * 128]
                if nmask:
                    zc = dpool.tile([128, nmask], F32, tag="zc", name=f"zc{s}")
                    nc.vector.tensor_copy(zc, it[:, W - nmask : W])

                ngroups = (T + GSZ - 1) // GSZ
                for qc in range(2):
                    qt = it[:, qc * QCHUNK : (qc + 1) * QCHUNK]
                    po = ps_o.tile([128, QCHUNK], F32, tag="po", name=f"po{s}_{qc}")
                    pd = ps_d.tile([128, QCHUNK], F32, tag="pd", name=f"pd{s}_{qc}")
                    pending = None
                    for g in range(ngroups + 1):
                        if g < ngroups:
                            gtiles = list(range(g * GSZ, min(g * GSZ + GSZ, T)))
                            gn = len(gtiles)
                            pss = ps_s.tile([128, GSZ * QCHUNK], F32, tag="ps")
                            for j, t in enumerate(gtiles):
                                nc.tensor.matmul(
                                    pss[:, j * QCHUNK : (j + 1) * QCHUNK],
                                    kt[:, t * 128 : (t + 1) * 128],
                                    qt,
                                    start=True,
                                    stop=True,
                                )
                            eg = epool.tile([128, GSZ * QCHUNK], BF16, tag="eg")
                            nc.scalar.activation(
                                eg[:, : gn * QCHUNK],
                                pss[:, : gn * QCHUNK],
                                mybir.ActivationFunctionType.Exp,
                                scale=SCALE,
                            )
                            for j, t in enumerate(gtiles):
                                if t >= mn - 1:
                                    nc.vector.tensor_scalar_mul(
                                        eg[:, j * QCHUNK : (j + 1) * QCHUNK],
                                        eg[:, j * QCHUNK : (j + 1) * QCHUNK],
                                        zc[:, t - (mn - 1) : t - (mn - 1) + 1],
                                    )
                            cur = (g, gtiles, eg)
                        else:
                            cur = None
                        if pending is not None:
                            pg, ptiles, peg = pending
                            for j, t in enumerate(ptiles):
                                es = peg[:, j * QCHUNK : (j + 1) * QCHUNK]
                                nc.tensor.matmul(
                                    po,
                                    vt[:, t * 128 : (t + 1) * 128],
                                    es,
                                    start=(t == 0),
                                    stop=(t == T - 1),
                                )
                            if len(ptiles) == 2:
                                gsum = gpool.tile([128, QCHUNK], BF16, tag="gsum")
                                nc.vector.tensor_add(
                                    gsum, peg[:, 0:QCHUNK], peg[:, QCHUNK:]
                                )
                            else:
                                gsum = peg[:, 0:QCHUNK]
                            nc.tensor.matmul(
                                pd,
                                ones,
                                gsum,
                                start=(pg == 0),
                                stop=(pg == ngroups - 1),
                            )
                        pending = cur
                    o = 2 * s + qc
                    osb = opool.tile([128, QCHUNK], BF16, tag="osb")
                    nc.vector.tensor_copy(osb, po)
                    nc.sync.dma_start(out=outs[f"num{o}"][:, :], in_=osb)
                    dsb = dpool.tile([1, QCHUNK], F32, tag="dsb")
                    nc.vector.tensor_copy(dsb, pd[0:1, :])
                    nc.sync.dma_start(out=outs[f"den{o}"][:, :], in_=dsb)

    nc.finalize()
    return nc


def _get_program_km(extents, mins, repeat=1):
    key = (tuple(extents), tuple(mins), repeat)
    if key not in _programs:
        _programs[key] = _build_program(tuple(extents), tuple(mins), repeat)
    return _programs[key]


def _shard_plan(vl):
    tiles = [max(1, int(math.ceil(int(vl[b]) / 128.0))) for b in range(B)]
    shards = sorted(
        ((tiles[b], b, h) for b in range(B) for h in range(NHALF)),
        key=lambda x: (-x[0], x[1], x[2]),
    )
    extents = tuple(shards[s * NCORES][0] for s in range(NSLOTS))
    mins = tuple(shards[s * NCORES + NCORES - 1][0] for s in range(NSLOTS))
    return shards, extents, mins


def _make_in_maps(queries, keys, values, vl, shards, extents, mins):
    import ml_dtypes

    BF = ml_dtypes.bfloat16
    kcache = {}

    def kv(b, T):
        key = (b, T)
        if key not in kcache:
            n = int(vl[b])
            kt = keys[b, : T * 128].T.astype(BF)
            vt = (
                values[b, : T * 128]
                .reshape(T, 128, D)
                .transpose(1, 0, 2)
                .reshape(128, T * D)
                .astype(BF)
            )
            z = np.zeros((T * 128,), np.float32)
            z[:n] = 1.0
            z = np.ascontiguousarray(z.reshape(T, 128).T).astype(BF)
            kcache[key] = (kt, vt, z)
        return kcache[key]

    qtr = {}

    def qtb(b):
        if b not in qtr:
            qtr[b] = queries[b].T.astype(BF)
        return qtr[b]

    in_maps = [{} for _ in range(NCORES)]
    for s in range(NSLOTS):
        T = extents[s]
        nmask = T - max(0, mins[s] - 1)
        for c in range(NCORES):
            _, b, h = shards[s * NCORES + c]
            kt, vt, z = kv(b, T)
            qt = qtb(b)[:, h * HCHUNK : (h + 1) * HCHUNK]
            in_maps[c][f"in{s}"] = np.concatenate([qt, kt, vt, z[:, T - nmask :]], axis=1)
    return in_maps


def kernel(queries, keys, values, valid_lens):
    from concourse.bass_utils import run_bass_kernel_spmd

    queries = np.ascontiguousarray(queries, dtype=np.float32)
    keys = np.ascontiguousarray(keys, dtype=np.float32)
    values = np.ascontiguousarray(values, dtype=np.float32)
    vl = np.asarray(valid_lens).astype(np.int64).clip(1, L)
    assert queries.shape == (B, L, D), queries.shape

    shards, extents, mins = _shard_plan(vl)
    nc = _get_program_km(extents, mins, _REPEAT)
    in_maps = _make_in_maps(queries, keys, values, vl, shards, extents, mins)

    res = run_bass_kernel_spmd(nc, in_maps, core_ids=list(range(NCORES)), trace=_TRACE)
    globals()["_last_results"] = res

    out = np.empty((B, L, D), np.float32)
    for s in range(NSLOTS):
        for c in range(NCORES):
            _, b, h = shards[s * NCORES + c]
            r = res.results[c]
            for qc in range(2):
                o = 2 * s + qc
                num = r[f"num{o}"].astype(np.float32)
                den = r[f"den{o}"]
                lo = h * HCHUNK + qc * QCHUNK
                out[b, lo : lo + QCHUNK] = (num / den).T
    return out
